# revision 3
# baseline (speedup 1.0000x reference)
"""Trainium2 Bass kernel for nn_CIN_81544249082266 (CIN / xDeepFM cross network).

Pure data parallel over 8 NeuronCores: each core processes 1024 of the 8192
batch rows; filters and output weights are replicated. No cross-device
communication (the host concatenates the per-core [1024] score vectors).

Math (per sample b, embedding dim d in [0,16), fields F0=39):
  layer k: z[(i,j), (b,d)] = x0[i,(b,d)] * h_k[j,(b,d)];  curr = relu(F_k^T z)
  h_{k+1} = curr rows [0:64), direct_k = remaining rows
  score[b] = sum_{m,d} direct[m,(b,d)] * (1 + w_nn[m]) + b_nn

v2 layout/engine plan (everything transposed: free axis r=(b*16+d)):
  - L0 uses only the 741 strictly-upper pairs (i<j), host-packed into 6
    partition blocks: XI/XJ [768, N] fp16 in HBM; z8 = XI*XJ on the Pool
    engine with fp8e4 output; fp8 filters (x64) via DoubleRow matmuls.
  - L1/L2: broadcast-A x tiled-h; DVE computes z for blocks 0..17 in 4-block
    "quad" + one pair instruction (in1 = h with a stride-0 broadcast dim);
    blocks 18,19 go to the Pool engine in fp8 + DoubleRow, offloading both
    the DVE and the PE.
  - All filters x64 (fp8 range); relu applies scale 1/64.
  - direct outputs d0/d1 share one 128-row tile so the residual+w_nn score
    contraction is 4 matmuls per chunk instead of 6.
  - A-broadcast + XI/XJ DMAs ride the sync/scalar HW DGE queues (idle
    sequencers); the Pool engine spends its time on tensor_tensor.
"""

import numpy as np
import ml_dtypes
from contextlib import ExitStack

import concourse.bass as bass
import concourse.tile as tile
from concourse import bacc, mybir
from concourse.bass_utils import run_bass_kernel_spmd

F0 = 39
D = 16
B = 8192
NCORES = 8
BC = B // NCORES            # 1024 samples per core
N = BC * D                  # 16384 r-columns per core
CH = 1024                   # chunk of r processed per inner iteration
NCHUNK = N // CH            # 16
NBLK = 20                   # 40*64/128 c-blocks per layer (i padded to 40)
NP0 = 6                     # L0 pair blocks (741 pairs -> 768 slots)
SC = 64.0                   # filter scaling (fp8 range), undone in relu
FP16 = mybir.dt.float16
FP32 = mybir.dt.float32
FP8 = mybir.dt.float8e4

_BUILT = None


def _build_program():
    nc = bacc.Bacc(
        "TRN2",
        target_bir_lowering=False,
        debug=False,
        num_devices=NCORES,
    )

    x2_d = nc.dram_tensor("x2", [40, N], FP16, kind="ExternalInput").ap()
    xi_d = nc.dram_tensor("xi", [NP0 * 128, N], FP16, kind="ExternalInput").ap()
    xj_d = nc.dram_tensor("xj", [NP0 * 128, N], FP16, kind="ExternalInput").ap()
    f0_d = nc.dram_tensor("f0", [128, NP0 * 128], FP8, kind="ExternalInput").ap()
    f_d = [
        nc.dram_tensor(f"f{k}", [128, 18 * 128], FP16, kind="ExternalInput").ap()
        for k in (1, 2)
    ]
    f8_d = [
        nc.dram_tensor(f"f{k}b8", [128, 2 * 128], FP8, kind="ExternalInput").ap()
        for k in (1, 2)
    ]
    wv_d = nc.dram_tensor("wv", [128, 2], FP16, kind="ExternalInput").ap()
    bias_d = nc.dram_tensor("bias", [1, 1], FP32, kind="ExternalInput").ap()
    out_d = nc.dram_tensor("out", [1, BC], FP32, kind="ExternalOutput").ap()

    relu = mybir.ActivationFunctionType.Relu
    mult = mybir.AluOpType.mult
    DR = mybir.MatmulPerfMode.DoubleRow

    with tile.TileContext(nc) as tc, ExitStack() as ctx:
        const = ctx.enter_context(tc.tile_pool(name="const", bufs=1))
        pool_a = ctx.enter_context(tc.tile_pool(name="a", bufs=10))
        pool_x = ctx.enter_context(tc.tile_pool(name="x", bufs=2))
        pool_z = ctx.enter_context(tc.tile_pool(name="z", bufs=3))
        pool_zp = ctx.enter_context(tc.tile_pool(name="zp", bufs=2))
        pool_z8 = ctx.enter_context(tc.tile_pool(name="z8", bufs=5))
        pool_h = ctx.enter_context(tc.tile_pool(name="h", bufs=3))
        pool_dt = ctx.enter_context(tc.tile_pool(name="dt", bufs=3))
        pool_r2 = ctx.enter_context(tc.tile_pool(name="r2", bufs=3))
        ps_curr = ctx.enter_context(tc.tile_pool(name="pcur", bufs=3, space="PSUM"))
        ps_s = ctx.enter_context(tc.tile_pool(name="ps", bufs=2, space="PSUM"))

        # --- resident constants ---
        f0sb = const.tile([128, NP0 * 128], FP8, tag="f0")
        nc.sync.dma_start(f0sb[:], f0_d[:])
        fsb, f8sb = [], []
        for k in range(2):
            f = const.tile([128, 18 * 128], FP16, tag=f"f{k + 1}", name=f"f{k + 1}")
            nc.sync.dma_start(f[:, : 18 * 64], f_d[k][:, : 18 * 64])
            nc.scalar.dma_start(f[:, 18 * 64 :], f_d[k][:, 18 * 64 :])
            fsb.append(f)
            f8 = const.tile([128, 2 * 128], FP8, tag=f"f8{k + 1}", name=f"f8{k + 1}")
            nc.sync.dma_start(f8[:], f8_d[k][:])
            f8sb.append(f8)
        wv = const.tile([128, 2], FP16)
        nc.sync.dma_start(wv[:], wv_d[:])
        bias = const.tile([1, 1], FP32)
        nc.sync.dma_start(bias[:], bias_d[:])
        scores = const.tile([1, BC], FP32)

        def load_chunk(c):
            """5 a-quad tiles via 20 broadcast DMAs + 2 fused XI/XJ loads,
            all on the sync/scalar HW DGE queues."""
            sl = slice(c * CH, (c + 1) * CH)
            a_quads = []
            for q in range(5):
                aq = pool_a.tile([128, 4 * CH], FP16, tag="a", name=f"a_{c}_{q}")
                for j in range(4):
                    k = 4 * q + j
                    src = x2_d[2 * k : 2 * k + 2, None, sl].to_broadcast([2, 64, CH])
                    eng = nc.sync if (k % 2 == 0) else nc.scalar
                    eng.dma_start(aq[:, j * CH : (j + 1) * CH], src)
                a_quads.append(aq)
            xia = pool_x.tile([128, NP0 * CH], FP16, tag="xia", name=f"xia_{c}")
            xja = pool_x.tile([128, NP0 * CH], FP16, tag="xja", name=f"xja_{c}")
            src_i = xi_d[:, sl].rearrange("(b p) c -> p b c", p=128)
            src_j = xj_d[:, sl].rearrange("(b p) c -> p b c", p=128)
            nc.sync.dma_start(xia[:].rearrange("p (b c) -> p b c", b=NP0), src_i)
            nc.scalar.dma_start(xja[:].rearrange("p (b c) -> p b c", b=NP0), src_j)
            return a_quads, xia, xja

        def layer_pass(c, layer, a_quads, h_op, fw, f8w):
            """L1/L2: 4 DVE quads + 1 DVE pair (blocks 0-17, fp16) and one
            Pool fp8 pair (blocks 18-19, DoubleRow)."""
            cur = ps_curr.tile([128, CH], FP32, tag="cur", name=f"cur_{c}_{layer}")
            h_b4 = h_op[:, None, :].to_broadcast([128, 4, CH])
            h_b2 = h_op[:, None, :].to_broadcast([128, 2, CH])
            z8 = pool_z8.tile([128, 2 * CH], FP8, tag="z8", name=f"z8_{c}_{layer}")
            nc.gpsimd.tensor_tensor(
                out=z8[:].rearrange("p (b c) -> p b c", b=2),
                in0=a_quads[4][:, 2 * CH :].rearrange("p (b c) -> p b c", b=2),
                in1=h_b2,
                op=mult,
            )
            for q in range(4):
                z = pool_z.tile([128, 4 * CH], FP16, tag="z", name=f"z_{c}_{layer}_{q}")
                nc.vector.tensor_tensor(
                    out=z[:].rearrange("p (b c) -> p b c", b=4),
                    in0=a_quads[q][:].rearrange("p (b c) -> p b c", b=4),
                    in1=h_b4,
                    op=mult,
                )
                for j in range(4):
                    k = 4 * q + j
                    for sgn in range(2):
                        nc.tensor.matmul(
                            cur[:, sgn * 512 : (sgn + 1) * 512],
                            lhsT=fw[:, k * 128 : (k + 1) * 128],
                            rhs=z[:, j * CH + sgn * 512 : j * CH + sgn * 512 + 512],
                            start=(k == 0),
                            stop=False,
                        )
            zp = pool_zp.tile([128, 2 * CH], FP16, tag="zp", name=f"zp_{c}_{layer}")
            nc.vector.tensor_tensor(
                out=zp[:].rearrange("p (b c) -> p b c", b=2),
                in0=a_quads[4][:, : 2 * CH].rearrange("p (b c) -> p b c", b=2),
                in1=h_b2,
                op=mult,
            )
            for j in range(2):
                k = 16 + j
                for sgn in range(2):
                    nc.tensor.matmul(
                        cur[:, sgn * 512 : (sgn + 1) * 512],
                        lhsT=fw[:, k * 128 : (k + 1) * 128],
                        rhs=zp[:, j * CH + sgn * 512 : j * CH + sgn * 512 + 512],
                        start=False, stop=False,
                    )
            zr = z8[:].rearrange("p (two c) -> p two c", two=2)
            lw = f8w[:].rearrange("p (two m) -> p two m", two=2)
            for sgn in range(2):
                nc.tensor.matmul(
                    cur[:, sgn * 512 : (sgn + 1) * 512],
                    lhsT=lw,
                    rhs=zr[:, :, sgn * 512 : (sgn + 1) * 512],
                    start=False, stop=True,
                    perf_mode=DR,
                )
            return cur

        def score_mms(sab, col, rhs_t, start, stop):
            for sgn in range(2):
                ssl = slice(sgn * 512, (sgn + 1) * 512)
                nc.tensor.matmul(
                    sab[32 * sgn : 32 * sgn + 1, :],
                    lhsT=wv[:, col : col + 1],
                    rhs=rhs_t[:, ssl],
                    start=start, stop=stop,
                    tile_position=(0, 32 * sgn),
                )

        def do_l0(c, xia, xja):
            """L0: 3 Pool pair-instructions make fp8 z; DoubleRow matmuls."""
            cur = ps_curr.tile([128, CH], FP32, tag="cur", name=f"cur0_{c}")
            z8s = []
            for q in range(3):
                z8 = pool_z8.tile([128, 2 * CH], FP8, tag="z8", name=f"z80_{c}_{q}")
                sl2 = slice(2 * q * CH, (2 * q + 2) * CH)
                nc.gpsimd.tensor_tensor(
                    out=z8[:], in0=xia[:, sl2], in1=xja[:, sl2], op=mult,
                )
                z8s.append(z8)
            for q in range(3):
                lw = f0sb[:, q * 256 : (q + 1) * 256].rearrange(
                    "p (two m) -> p two m", two=2
                )
                zr = z8s[q][:].rearrange("p (two c) -> p two c", two=2)
                for sgn in range(2):
                    nc.tensor.matmul(
                        cur[:, sgn * 512 : (sgn + 1) * 512],
                        lhsT=lw,
                        rhs=zr[:, :, sgn * 512 : (sgn + 1) * 512],
                        start=(q == 0), stop=(q == 2),
                        perf_mode=DR,
                    )
            h_t = pool_h.tile([128, CH], FP16, tag="h", name=f"h_{c}")
            d01 = pool_dt.tile([128, CH], FP16, tag="d", name=f"d_{c}")
            nc.scalar.activation(h_t[0:64, :], cur[0:64, :], relu, scale=1.0 / SC)
            nc.scalar.activation(h_t[64:128, :], cur[0:64, :], relu, scale=1.0 / SC)
            nc.scalar.activation(d01[0:64, :], cur[64:128, :], relu, scale=1.0 / SC)
            return h_t, d01

        def emit_reduces(t, sab_t):
            for sgn in range(2):
                off = t * (CH // D) + sgn * 32
                nc.vector.tensor_reduce(
                    out=scores[0:1, off : off + 32],
                    in_=sab_t[32 * sgn : 32 * sgn + 1, :].rearrange(
                        "p (g x) -> p g x", x=D
                    ),
                    axis=mybir.AxisListType.X,
                    op=mybir.AluOpType.add,
                )

        chunks = {}
        for cc in range(3):
            chunks[cc] = load_chunk(cc)
        state = do_l0(0, chunks[0][1], chunks[0][2])

        pending_reduce = None
        for t in range(NCHUNK):
            if t + 3 < NCHUNK:
                chunks[t + 3] = load_chunk(t + 3)
            a_quads, _, _ = chunks[t]
            h1, d01 = state
            cur1 = layer_pass(t, 1, a_quads, h1, fsb[0], f8sb[0])
            if pending_reduce is not None:
                pt, psab, pr2 = pending_reduce
                score_mms(psab, 1, pr2, start=False, stop=True)
                emit_reduces(pt, psab)
            h2 = pool_h.tile([128, CH], FP16, tag="h", name=f"h2_{t}")
            nc.scalar.activation(h2[0:64, :], cur1[0:64, :], relu, scale=1.0 / SC)
            nc.scalar.activation(h2[64:128, :], cur1[0:64, :], relu, scale=1.0 / SC)
            nc.scalar.activation(d01[64:128, :], cur1[64:128, :], relu, scale=1.0 / SC)
            if t + 1 < NCHUNK:
                state = do_l0(t + 1, chunks[t + 1][1], chunks[t + 1][2])
            sab = ps_s.tile([33, 512], FP32, tag="sab", name=f"sab_{t}")
            score_mms(sab, 0, d01, start=True, stop=False)
            cur2 = layer_pass(t, 2, a_quads, h2, fsb[1], f8sb[1])
            r2 = pool_r2.tile([128, CH], FP16, tag="r2", name=f"r2_{t}")
            nc.scalar.activation(r2[:], cur2[:], relu, scale=1.0 / SC)
            del chunks[t]
            pending_reduce = (t, sab, r2)

        pt, psab, pr2 = pending_reduce
        score_mms(psab, 1, pr2, start=False, stop=True)
        emit_reduces(pt, psab)
        nc.vector.tensor_scalar_add(scores[:], scores[:], bias[0:1, 0:1])
        nc.sync.dma_start(out_d[:], scores[:])

    nc.compile()
    return nc


def _prep_inputs(nn_input, f0, f1, f2, w_nn, b_nn):
    """Host-side preprocessing into the kernel's layouts."""
    nn_input = np.asarray(nn_input, dtype=np.float32)
    f0 = np.asarray(f0, dtype=np.float32)
    f1 = np.asarray(f1, dtype=np.float32)
    f2 = np.asarray(f2, dtype=np.float32)
    w_nn = np.asarray(w_nn, dtype=np.float32).reshape(-1)
    b_nn = np.asarray(b_nn, dtype=np.float32).reshape(-1)

    # L1/L2 filters: [39*64, 128] i-major, x64; blocks 0..17 fp16 lhsT,
    # blocks 18,19 fp8 lhsT (DoubleRow pair)
    def pack(f):
        out = np.zeros((NBLK * 128, 128), np.float32)
        out[: F0 * 64] = SC * f
        blocks = out.reshape(NBLK, 128, 128).transpose(1, 0, 2)  # [128, blk, 128]
        f16p = np.ascontiguousarray(blocks[:, :18].reshape(128, 18 * 128)).astype(
            np.float16
        )
        f8p = np.ascontiguousarray(blocks[:, 18:].reshape(128, 2 * 128)).astype(
            ml_dtypes.float8_e4m3
        )
        return f16p, f8p

    f1p, f1b8 = pack(f1)
    f2p, f2b8 = pack(f2)

    # L0: strictly-upper pairs packed; filter x2 (sym) x64, fp8 lhsT blocks
    iu, ju = np.triu_indices(F0, k=1)
    f0r = f0.reshape(F0, F0, 128)
    w0 = np.zeros((NP0 * 128, 128), np.float32)
    w0[: len(iu)] = 2.0 * SC * f0r[iu, ju]
    w0b = w0.reshape(NP0, 128, 128).transpose(1, 0, 2).reshape(128, NP0 * 128)
    f0p = np.ascontiguousarray(w0b).astype(ml_dtypes.float8_e4m3)

    wv = np.zeros((128, 2), np.float32)
    wv[0:64, 0] = 1.0 + w_nn[0:64]
    wv[64:128, 0] = 1.0 + w_nn[64:128]
    wv[:, 1] = 1.0 + w_nn[128:256]
    wv = wv.astype(np.float16)
    bias = b_nn.reshape(1, 1).astype(np.float32)

    x0 = nn_input.reshape(B, F0, D)
    in_maps = []
    for cidx in range(NCORES):
        xc = x0[cidx * BC : (cidx + 1) * BC]            # [BC, 39, 16]
        xt = xc.transpose(1, 0, 2).reshape(F0, N).astype(np.float16)
        x2h = np.zeros((40, N), np.float16)
        x2h[:F0] = xt
        xi = np.zeros((NP0 * 128, N), np.float16)
        xj = np.zeros((NP0 * 128, N), np.float16)
        xi[: len(iu)] = xt[iu]
        xj[: len(ju)] = xt[ju]
        in_maps.append(
            {"x2": x2h, "xi": xi, "xj": xj, "f0": f0p,
             "f1": f1p, "f2": f2p, "f1b8": f1b8, "f2b8": f2b8,
             "wv": wv, "bias": bias}
        )
    return in_maps


def _run(inputs, trace=False, trace_kwargs=None):
    global _BUILT
    if _BUILT is None:
        _BUILT = _build_program()
    nc = _BUILT
    in_maps = _prep_inputs(**inputs)
    res = run_bass_kernel_spmd(
        nc,
        in_maps,
        core_ids=list(range(NCORES)),
        trace=trace,
        **(trace_kwargs or {}),
    )
    out = np.concatenate(
        [res.results[c]["out"].reshape(BC) for c in range(NCORES)]
    )
    return out.reshape(B, 1).astype(np.float32), res


def kernel(**inputs):
    out, _ = _run(inputs)
    return out


# revision 6
# speedup vs baseline: 2.2625x; 2.2625x over previous
"""Trainium2 Bass kernel for nn_CIN_81544249082266 (CIN / xDeepFM cross network).

Pure data parallel over 8 NeuronCores: each core processes 1024 of the 8192
batch rows; filters and output weights are replicated. No cross-device
communication (the host concatenates the per-core [1024] score vectors).

Math (per sample b, embedding dim d in [0,16), fields F0=39):
  layer k: z[(i,j), (b,d)] = x0[i,(b,d)] * h_k[j,(b,d)];  curr = relu(F_k^T z)
  h_{k+1} = curr rows [0:64), direct_k = remaining rows
  score[b] = sum_{m,d} direct[m,(b,d)] * (1 + w_nn[m]) + b_nn

v2 layout/engine plan (everything transposed: free axis r=(b*16+d)):
  - L0 uses only the 741 strictly-upper pairs (i<j), host-packed into 6
    partition blocks: XI/XJ [768, N] fp16 in HBM; z8 = XI*XJ on the Pool
    engine with fp8e4 output; fp8 filters (x64) via DoubleRow matmuls.
  - L1/L2: broadcast-A x tiled-h; DVE computes z for blocks 0..17 in 4-block
    "quad" + one pair instruction (in1 = h with a stride-0 broadcast dim);
    blocks 18,19 go to the Pool engine in fp8 + DoubleRow, offloading both
    the DVE and the PE.
  - All filters x64 (fp8 range); relu applies scale 1/64.
  - direct outputs d0/d1 share one 128-row tile so the residual+w_nn score
    contraction is 4 matmuls per chunk instead of 6.
  - A-broadcast + XI/XJ DMAs ride the sync/scalar HW DGE queues (idle
    sequencers); the Pool engine spends its time on tensor_tensor.
"""

import numpy as np
import ml_dtypes
from contextlib import ExitStack

import concourse.bass as bass
import concourse.tile as tile
from concourse import bacc, mybir
from concourse.bass_utils import run_bass_kernel_spmd

F0 = 39
D = 16
B = 8192
NCORES = 8
BC = B // NCORES            # 1024 samples per core
N = BC * D                  # 16384 r-columns per core
CH = 1024                   # chunk of r processed per inner iteration
NCHUNK = N // CH            # 16
NBLK = 20                   # 40*64/128 c-blocks per layer (i padded to 40)
NP0 = 6                     # L0 pair blocks (741 pairs -> 768 slots)
SC = 64.0                   # filter scaling (fp8 range), undone in relu
FP16 = mybir.dt.float16
FP32 = mybir.dt.float32
FP8 = mybir.dt.float8e4

_BUILT = None


def _build_program():
    nc = bacc.Bacc(
        "TRN2",
        target_bir_lowering=False,
        debug=False,
        num_devices=NCORES,
    )

    x2_d = nc.dram_tensor("x2", [40, N], FP16, kind="ExternalInput").ap()
    xi_d = nc.dram_tensor("xi", [NP0 * 128, N], FP16, kind="ExternalInput").ap()
    xj_d = nc.dram_tensor("xj", [NP0 * 128, N], FP16, kind="ExternalInput").ap()
    f0_d = nc.dram_tensor("f0", [128, NP0 * 128], FP8, kind="ExternalInput").ap()
    f_d = [
        nc.dram_tensor(f"f{k}", [128, NBLK * 128], FP16, kind="ExternalInput").ap()
        for k in (1, 2)
    ]
    wv_d = nc.dram_tensor("wv", [128, 2], FP16, kind="ExternalInput").ap()
    bias_d = nc.dram_tensor("bias", [1, 1], FP32, kind="ExternalInput").ap()
    out_d = nc.dram_tensor("out", [1, BC], FP32, kind="ExternalOutput").ap()

    relu = mybir.ActivationFunctionType.Relu
    mult = mybir.AluOpType.mult
    DR = mybir.MatmulPerfMode.DoubleRow

    with tile.TileContext(nc) as tc, ExitStack() as ctx:
        const = ctx.enter_context(tc.tile_pool(name="const", bufs=1))
        pool_a = ctx.enter_context(tc.tile_pool(name="a", bufs=2))
        pool_x = ctx.enter_context(tc.tile_pool(name="x", bufs=2))
        pool_z = ctx.enter_context(tc.tile_pool(name="z", bufs=3))
        pool_z8 = ctx.enter_context(tc.tile_pool(name="z8", bufs=5))
        pool_h = ctx.enter_context(tc.tile_pool(name="h", bufs=3))
        pool_dt = ctx.enter_context(tc.tile_pool(name="dt", bufs=3))
        pool_r2 = ctx.enter_context(tc.tile_pool(name="r2", bufs=3))
        ps_curr = ctx.enter_context(tc.tile_pool(name="pcur", bufs=3, space="PSUM"))
        ps_s = ctx.enter_context(tc.tile_pool(name="ps", bufs=2, space="PSUM"))

        # --- resident constants ---
        f0sb = const.tile([128, NP0 * 128], FP8, tag="f0")
        nc.sync.dma_start(f0sb[:], f0_d[:])
        fsb = []
        for k in range(2):
            f = const.tile([128, NBLK * 128], FP16, tag=f"f{k + 1}", name=f"f{k + 1}")
            nc.sync.dma_start(f[:, : NBLK * 64], f_d[k][:, : NBLK * 64])
            nc.scalar.dma_start(f[:, NBLK * 64 :], f_d[k][:, NBLK * 64 :])
            fsb.append(f)
        wv = const.tile([128, 2], FP16)
        nc.sync.dma_start(wv[:], wv_d[:])
        bias = const.tile([1, 1], FP32)
        nc.sync.dma_start(bias[:], bias_d[:])
        scores = const.tile([1, BC], FP32)

        def load_chunk(c):
            """5 a-quad tiles via 20 broadcast DMAs + 2 fused XI/XJ loads,
            all on the sync/scalar HW DGE queues."""
            sl = slice(c * CH, (c + 1) * CH)
            a_all = pool_a.tile([128, NBLK * CH], FP16, tag="a", name=f"a_{c}")
            # partition p = two*64 + b holds x row 2j+two broadcast over b;
            # one fused SW-DGE chain per row parity (3-dim APs, no balancing)
            rows = x2_d[:, sl].rearrange("(j two) c -> two j c", two=2)
            for a in range(2):
                src = rows[a : a + 1].to_broadcast([64, NBLK, CH])
                nc.gpsimd.dma_start(
                    a_all[64 * a : 64 * (a + 1)].rearrange(
                        "p (j c) -> p j c", j=NBLK
                    ),
                    src,
                )
            xia = pool_x.tile([128, NP0 * CH], FP16, tag="xia", name=f"xia_{c}")
            xja = pool_x.tile([128, NP0 * CH], FP16, tag="xja", name=f"xja_{c}")
            src_i = xi_d[:, sl].rearrange("(b p) c -> p b c", p=128)
            src_j = xj_d[:, sl].rearrange("(b p) c -> p b c", p=128)
            nc.gpsimd.dma_start(xia[:].rearrange("p (b c) -> p b c", b=NP0), src_i)
            nc.gpsimd.dma_start(xja[:].rearrange("p (b c) -> p b c", b=NP0), src_j)
            return a_all, xia, xja

        def layer_pass(c, layer, a_all, h_op, fw):
            """L1/L2: 5 DVE quad z instructions feeding 8 matmuls each."""
            cur = ps_curr.tile([128, CH], FP32, tag="cur", name=f"cur_{c}_{layer}")
            h_b4 = h_op[:, None, :].to_broadcast([128, 4, CH])
            for q in range(5):
                z = pool_z.tile([128, 4 * CH], FP16, tag="z", name=f"z_{c}_{layer}_{q}")
                nc.vector.tensor_tensor(
                    out=z[:].rearrange("p (b c) -> p b c", b=4),
                    in0=a_all[:, 4 * q * CH : (4 * q + 4) * CH].rearrange(
                        "p (b c) -> p b c", b=4
                    ),
                    in1=h_b4,
                    op=mult,
                )
                for j in range(4):
                    k = 4 * q + j
                    for sgn in range(2):
                        nc.tensor.matmul(
                            cur[:, sgn * 512 : (sgn + 1) * 512],
                            lhsT=fw[:, k * 128 : (k + 1) * 128],
                            rhs=z[:, j * CH + sgn * 512 : j * CH + sgn * 512 + 512],
                            start=(k == 0),
                            stop=(k == NBLK - 1),
                        )
            return cur

        def score_mms(sab, col, rhs_t, start, stop):
            for sgn in range(2):
                ssl = slice(sgn * 512, (sgn + 1) * 512)
                nc.tensor.matmul(
                    sab[32 * sgn : 32 * sgn + 1, :],
                    lhsT=wv[:, col : col + 1],
                    rhs=rhs_t[:, ssl],
                    start=start, stop=stop,
                    tile_position=(0, 32 * sgn),
                )

        def do_l0(c, xia, xja):
            """L0: 3 Pool pair-instructions make fp8 z; DoubleRow matmuls."""
            cur = ps_curr.tile([128, CH], FP32, tag="cur", name=f"cur0_{c}")
            z8s = []
            for q in range(3):
                z8 = pool_z8.tile([128, 2 * CH], FP8, tag="z8", name=f"z80_{c}_{q}")
                sl2 = slice(2 * q * CH, (2 * q + 2) * CH)
                nc.gpsimd.tensor_tensor(
                    out=z8[:], in0=xia[:, sl2], in1=xja[:, sl2], op=mult,
                )
                z8s.append(z8)
            for q in range(3):
                lw = f0sb[:, q * 256 : (q + 1) * 256].rearrange(
                    "p (two m) -> p two m", two=2
                )
                zr = z8s[q][:].rearrange("p (two c) -> p two c", two=2)
                for sgn in range(2):
                    nc.tensor.matmul(
                        cur[:, sgn * 512 : (sgn + 1) * 512],
                        lhsT=lw,
                        rhs=zr[:, :, sgn * 512 : (sgn + 1) * 512],
                        start=(q == 0), stop=(q == 2),
                        perf_mode=DR,
                    )
            h_t = pool_h.tile([128, CH], FP16, tag="h", name=f"h_{c}")
            d01 = pool_dt.tile([128, CH], FP16, tag="d", name=f"d_{c}")
            nc.scalar.activation(h_t[0:64, :], cur[0:64, :], relu, scale=1.0 / SC)
            nc.scalar.activation(h_t[64:128, :], cur[0:64, :], relu, scale=1.0 / SC)
            nc.scalar.activation(d01[0:64, :], cur[64:128, :], relu, scale=1.0 / SC)
            return h_t, d01

        def emit_reduces(t, sab_t):
            for sgn in range(2):
                off = t * (CH // D) + sgn * 32
                nc.vector.tensor_reduce(
                    out=scores[0:1, off : off + 32],
                    in_=sab_t[32 * sgn : 32 * sgn + 1, :].rearrange(
                        "p (g x) -> p g x", x=D
                    ),
                    axis=mybir.AxisListType.X,
                    op=mybir.AluOpType.add,
                )

        chunks = {}
        for cc in range(3):
            chunks[cc] = load_chunk(cc)
        state = do_l0(0, chunks[0][1], chunks[0][2])

        pending_reduce = None
        for t in range(NCHUNK):
            if t + 3 < NCHUNK:
                chunks[t + 3] = load_chunk(t + 3)
            a_all, _, _ = chunks[t]
            h1, d01 = state
            cur1 = layer_pass(t, 1, a_all, h1, fsb[0])
            if pending_reduce is not None:
                pt, psab, pr2 = pending_reduce
                score_mms(psab, 1, pr2, start=False, stop=True)
                emit_reduces(pt, psab)
            h2 = pool_h.tile([128, CH], FP16, tag="h", name=f"h2_{t}")
            nc.scalar.activation(h2[0:64, :], cur1[0:64, :], relu, scale=1.0 / SC)
            nc.scalar.activation(h2[64:128, :], cur1[0:64, :], relu, scale=1.0 / SC)
            nc.scalar.activation(d01[64:128, :], cur1[64:128, :], relu, scale=1.0 / SC)
            if t + 1 < NCHUNK:
                state = do_l0(t + 1, chunks[t + 1][1], chunks[t + 1][2])
            sab = ps_s.tile([33, 512], FP32, tag="sab", name=f"sab_{t}")
            score_mms(sab, 0, d01, start=True, stop=False)
            cur2 = layer_pass(t, 2, a_all, h2, fsb[1])
            r2 = pool_r2.tile([128, CH], FP16, tag="r2", name=f"r2_{t}")
            nc.scalar.activation(r2[:], cur2[:], relu, scale=1.0 / SC)
            del chunks[t]
            pending_reduce = (t, sab, r2)

        pt, psab, pr2 = pending_reduce
        score_mms(psab, 1, pr2, start=False, stop=True)
        emit_reduces(pt, psab)
        nc.vector.tensor_scalar_add(scores[:], scores[:], bias[0:1, 0:1])
        nc.sync.dma_start(out_d[:], scores[:])

    nc.compile()
    return nc


def _prep_inputs(nn_input, f0, f1, f2, w_nn, b_nn):
    """Host-side preprocessing into the kernel's layouts."""
    nn_input = np.asarray(nn_input, dtype=np.float32)
    f0 = np.asarray(f0, dtype=np.float32)
    f1 = np.asarray(f1, dtype=np.float32)
    f2 = np.asarray(f2, dtype=np.float32)
    w_nn = np.asarray(w_nn, dtype=np.float32).reshape(-1)
    b_nn = np.asarray(b_nn, dtype=np.float32).reshape(-1)

    # L1/L2 filters: [39*64, 128] i-major, x64, fp16 lhsT blocks
    def pack(f):
        out = np.zeros((NBLK * 128, 128), np.float32)
        out[: F0 * 64] = SC * f
        blocks = out.reshape(NBLK, 128, 128).transpose(1, 0, 2)
        return np.ascontiguousarray(blocks.reshape(128, NBLK * 128)).astype(
            np.float16
        )

    f1p, f2p = pack(f1), pack(f2)

    # L0: strictly-upper pairs packed; filter x2 (sym) x64, fp8 lhsT blocks
    iu, ju = np.triu_indices(F0, k=1)
    f0r = f0.reshape(F0, F0, 128)
    w0 = np.zeros((NP0 * 128, 128), np.float32)
    w0[: len(iu)] = 2.0 * SC * f0r[iu, ju]
    w0b = w0.reshape(NP0, 128, 128).transpose(1, 0, 2).reshape(128, NP0 * 128)
    f0p = np.ascontiguousarray(w0b).astype(ml_dtypes.float8_e4m3)

    wv = np.zeros((128, 2), np.float32)
    wv[0:64, 0] = 1.0 + w_nn[0:64]
    wv[64:128, 0] = 1.0 + w_nn[64:128]
    wv[:, 1] = 1.0 + w_nn[128:256]
    wv = wv.astype(np.float16)
    bias = b_nn.reshape(1, 1).astype(np.float32)

    x0 = nn_input.reshape(B, F0, D)
    in_maps = []
    for cidx in range(NCORES):
        xc = x0[cidx * BC : (cidx + 1) * BC]            # [BC, 39, 16]
        xt = xc.transpose(1, 0, 2).reshape(F0, N).astype(np.float16)
        x2h = np.zeros((40, N), np.float16)
        x2h[:F0] = xt
        xi = np.zeros((NP0 * 128, N), np.float16)
        xj = np.zeros((NP0 * 128, N), np.float16)
        xi[: len(iu)] = xt[iu]
        xj[: len(ju)] = xt[ju]
        in_maps.append(
            {"x2": x2h, "xi": xi, "xj": xj, "f0": f0p,
             "f1": f1p, "f2": f2p, "wv": wv, "bias": bias}
        )
    return in_maps


def _run(inputs, trace=False, trace_kwargs=None):
    global _BUILT
    if _BUILT is None:
        _BUILT = _build_program()
    nc = _BUILT
    in_maps = _prep_inputs(**inputs)
    res = run_bass_kernel_spmd(
        nc,
        in_maps,
        core_ids=list(range(NCORES)),
        trace=trace,
        **(trace_kwargs or {}),
    )
    out = np.concatenate(
        [res.results[c]["out"].reshape(BC) for c in range(NCORES)]
    )
    return out.reshape(B, 1).astype(np.float32), res


def kernel(**inputs):
    out, _ = _run(inputs)
    return out


# revision 8
# speedup vs baseline: 2.4229x; 1.0709x over previous
"""Trainium2 Bass kernel for nn_CIN_81544249082266 (CIN / xDeepFM cross network).

Pure data parallel over 8 NeuronCores: each core processes 1024 of the 8192
batch rows; filters and output weights are replicated. No cross-device
communication (the host concatenates the per-core [1024] score vectors).

Math (per sample b, embedding dim d in [0,16), fields F0=39):
  layer k: z[(i,j), (b,d)] = x0[i,(b,d)] * h_k[j,(b,d)];  curr = relu(F_k^T z)
  h_{k+1} = curr rows [0:64), direct_k = remaining rows
  score[b] = sum_{m,d} direct[m,(b,d)] * (1 + w_nn[m]) + b_nn

v2 layout/engine plan (everything transposed: free axis r=(b*16+d)):
  - L0 uses only the 741 strictly-upper pairs (i<j), host-packed into 6
    partition blocks: XI/XJ [768, N] fp16 in HBM; z8 = XI*XJ on the Pool
    engine with fp8e4 output; fp8 filters (x64) via DoubleRow matmuls.
  - L1/L2: broadcast-A x tiled-h; DVE computes z for blocks 0..17 in 4-block
    "quad" + one pair instruction (in1 = h with a stride-0 broadcast dim);
    blocks 18,19 go to the Pool engine in fp8 + DoubleRow, offloading both
    the DVE and the PE.
  - All filters x64 (fp8 range); relu applies scale 1/64.
  - direct outputs d0/d1 share one 128-row tile so the residual+w_nn score
    contraction is 4 matmuls per chunk instead of 6.
  - A-broadcast + XI/XJ DMAs ride the sync/scalar HW DGE queues (idle
    sequencers); the Pool engine spends its time on tensor_tensor.
"""

import numpy as np
import ml_dtypes
from contextlib import ExitStack

import concourse.bass as bass
import concourse.tile as tile
from concourse import bacc, mybir
from concourse.bass_utils import run_bass_kernel_spmd

F0 = 39
D = 16
B = 8192
NCORES = 8
BC = B // NCORES            # 1024 samples per core
N = BC * D                  # 16384 r-columns per core
CH = 1024                   # chunk of r processed per inner iteration
NCHUNK = N // CH            # 16
NBLK = 20                   # 40*64/128 c-blocks per layer (i padded to 40)
NP0 = 6                     # L0 pair blocks (741 pairs -> 768 slots)
SC = 64.0                   # filter scaling (fp8 range), undone in relu
FP16 = mybir.dt.float16
FP32 = mybir.dt.float32
FP8 = mybir.dt.float8e4

_BUILT = None


def _build_program():
    nc = bacc.Bacc(
        "TRN2",
        target_bir_lowering=False,
        debug=False,
        num_devices=NCORES,
    )

    x2_d = nc.dram_tensor("x2", [40, N], FP16, kind="ExternalInput").ap()
    xi_d = nc.dram_tensor("xi", [NP0 * 128, N], FP16, kind="ExternalInput").ap()
    xj_d = nc.dram_tensor("xj", [NP0 * 128, N], FP16, kind="ExternalInput").ap()
    f0_d = nc.dram_tensor("f0", [128, NP0 * 128], FP8, kind="ExternalInput").ap()
    f_d = [
        nc.dram_tensor(f"f{k}", [128, NBLK * 128], FP16, kind="ExternalInput").ap()
        for k in (1, 2)
    ]
    wv_d = nc.dram_tensor("wv", [128, 2], FP16, kind="ExternalInput").ap()
    bias_d = nc.dram_tensor("bias", [1, 1], FP32, kind="ExternalInput").ap()
    out_d = nc.dram_tensor("out", [1, BC], FP32, kind="ExternalOutput").ap()

    relu = mybir.ActivationFunctionType.Relu
    mult = mybir.AluOpType.mult
    DR = mybir.MatmulPerfMode.DoubleRow

    with tile.TileContext(nc) as tc, ExitStack() as ctx:
        const = ctx.enter_context(tc.tile_pool(name="const", bufs=1))
        pool_a = ctx.enter_context(tc.tile_pool(name="a", bufs=2))
        pool_x = ctx.enter_context(tc.tile_pool(name="x", bufs=2))
        pool_z = ctx.enter_context(tc.tile_pool(name="z", bufs=4))
        pool_z8 = ctx.enter_context(tc.tile_pool(name="z8", bufs=4))
        pool_h = ctx.enter_context(tc.tile_pool(name="h", bufs=2))
        pool_dt = ctx.enter_context(tc.tile_pool(name="dt", bufs=2))
        pool_r2 = ctx.enter_context(tc.tile_pool(name="r2", bufs=2))
        ps_curr = ctx.enter_context(tc.tile_pool(name="pcur", bufs=3, space="PSUM"))
        ps_s = ctx.enter_context(tc.tile_pool(name="ps", bufs=2, space="PSUM"))

        # --- resident constants ---
        f0sb = const.tile([128, NP0 * 128], FP8, tag="f0")
        nc.sync.dma_start(f0sb[:], f0_d[:])
        fsb = []
        for k in range(2):
            f = const.tile([128, NBLK * 128], FP16, tag=f"f{k + 1}", name=f"f{k + 1}")
            nc.sync.dma_start(f[:, : NBLK * 64], f_d[k][:, : NBLK * 64])
            nc.scalar.dma_start(f[:, NBLK * 64 :], f_d[k][:, NBLK * 64 :])
            fsb.append(f)
        wv = const.tile([128, 2], FP16)
        nc.sync.dma_start(wv[:], wv_d[:])
        bias = const.tile([1, 1], FP32)
        nc.sync.dma_start(bias[:], bias_d[:])
        scores = const.tile([1, BC], FP32)

        def load_chunk(c):
            """5 a-quad tiles via 20 broadcast DMAs + 2 fused XI/XJ loads,
            all on the sync/scalar HW DGE queues."""
            sl = slice(c * CH, (c + 1) * CH)
            # XI/XJ first: L0(c) runs one iteration earlier than the a_all
            # consumers, and the SW-DGE ring drains strictly in issue order.
            xia = pool_x.tile([128, NP0 * CH], FP16, tag="xia", name=f"xia_{c}")
            xja = pool_x.tile([128, NP0 * CH], FP16, tag="xja", name=f"xja_{c}")
            src_i = xi_d[:, sl].rearrange("(b p) c -> p b c", p=128)
            src_j = xj_d[:, sl].rearrange("(b p) c -> p b c", p=128)
            nc.gpsimd.dma_start(xia[:].rearrange("p (b c) -> p b c", b=NP0), src_i)
            nc.gpsimd.dma_start(xja[:].rearrange("p (b c) -> p b c", b=NP0), src_j)
            a_all = pool_a.tile([128, NBLK * CH], FP16, tag="a", name=f"a_{c}")
            # partition p = two*64 + b holds x row 2j+two broadcast over b;
            # one fused SW-DGE chain per row parity (3-dim APs, no balancing)
            rows = x2_d[:, sl].rearrange("(j two) c -> two j c", two=2)
            for a in range(2):
                src = rows[a : a + 1].to_broadcast([64, NBLK, CH])
                nc.gpsimd.dma_start(
                    a_all[64 * a : 64 * (a + 1)].rearrange(
                        "p (j c) -> p j c", j=NBLK
                    ),
                    src,
                )
            return a_all, xia, xja

        def layer_pass(c, layer, a_all, h_op, fw):
            """L1/L2: 5 DVE quad z instructions feeding 8 matmuls each."""
            cur = ps_curr.tile([128, CH], FP32, tag="cur", name=f"cur_{c}_{layer}")
            h_b4 = h_op[:]
            for q in range(5):
                z = pool_z.tile([128, 4 * CH], FP16, tag="z", name=f"z_{c}_{layer}_{q}")
                nc.vector.tensor_tensor(
                    out=z[:].rearrange("p (b c) -> p b c", b=4),
                    in0=a_all[:, 4 * q * CH : (4 * q + 4) * CH].rearrange(
                        "p (b c) -> p b c", b=4
                    ),
                    in1=h_b4,
                    op=mult,
                )
                for j in range(4):
                    k = 4 * q + j
                    for sgn in range(2):
                        nc.tensor.matmul(
                            cur[:, sgn * 512 : (sgn + 1) * 512],
                            lhsT=fw[:, k * 128 : (k + 1) * 128],
                            rhs=z[:, j * CH + sgn * 512 : j * CH + sgn * 512 + 512],
                            start=(k == 0),
                            stop=(k == NBLK - 1),
                        )
            return cur

        def score_mms(sab, col, rhs_t, start, stop):
            for sgn in range(2):
                ssl = slice(sgn * 512, (sgn + 1) * 512)
                nc.tensor.matmul(
                    sab[32 * sgn : 32 * sgn + 1, :],
                    lhsT=wv[:, col : col + 1],
                    rhs=rhs_t[:, ssl],
                    start=start, stop=stop,
                    tile_position=(0, 32 * sgn),
                )

        def do_l0(c, xia, xja):
            """L0: 3 Pool pair-instructions make fp8 z; DoubleRow matmuls."""
            cur = ps_curr.tile([128, CH], FP32, tag="cur", name=f"cur0_{c}")
            z8s = []
            for q in range(3):
                z8 = pool_z8.tile([128, 2 * CH], FP8, tag="z8", name=f"z80_{c}_{q}")
                sl2 = slice(2 * q * CH, (2 * q + 2) * CH)
                nc.gpsimd.tensor_tensor(
                    out=z8[:], in0=xia[:, sl2], in1=xja[:, sl2], op=mult,
                )
                z8s.append(z8)
            for q in range(3):
                lw = f0sb[:, q * 256 : (q + 1) * 256].rearrange(
                    "p (two m) -> p two m", two=2
                )
                zr = z8s[q][:].rearrange("p (two c) -> p two c", two=2)
                for sgn in range(2):
                    nc.tensor.matmul(
                        cur[:, sgn * 512 : (sgn + 1) * 512],
                        lhsT=lw,
                        rhs=zr[:, :, sgn * 512 : (sgn + 1) * 512],
                        start=(q == 0), stop=(q == 2),
                        perf_mode=DR,
                    )
            h_t = pool_h.tile([128, 4 * CH], FP16, tag="h", name=f"h_{c}")
            d01 = pool_dt.tile([128, CH], FP16, tag="d", name=f"d_{c}")
            hr = h_t[:].rearrange("p (b c) -> p b c", b=4)
            nc.scalar.activation(
                hr[0:64], cur[0:64, None, :].to_broadcast([64, 4, CH]),
                relu, scale=1.0 / SC)
            nc.scalar.activation(
                hr[64:128], cur[0:64, None, :].to_broadcast([64, 4, CH]),
                relu, scale=1.0 / SC)
            nc.scalar.activation(d01[0:64, :], cur[64:128, :], relu, scale=1.0 / SC)
            return h_t, d01

        def emit_reduces(t, sab_t):
            for sgn in range(2):
                off = t * (CH // D) + sgn * 32
                nc.vector.tensor_reduce(
                    out=scores[0:1, off : off + 32],
                    in_=sab_t[32 * sgn : 32 * sgn + 1, :].rearrange(
                        "p (g x) -> p g x", x=D
                    ),
                    axis=mybir.AxisListType.X,
                    op=mybir.AluOpType.add,
                )

        chunks = {}
        for cc in range(3):
            chunks[cc] = load_chunk(cc)
        state = do_l0(0, chunks[0][1], chunks[0][2])

        pending_reduce = None
        for t in range(NCHUNK):
            if t + 3 < NCHUNK:
                chunks[t + 3] = load_chunk(t + 3)
            a_all, _, _ = chunks[t]
            h1, d01 = state
            cur1 = layer_pass(t, 1, a_all, h1, fsb[0])
            if pending_reduce is not None:
                pt, psab, pr2 = pending_reduce
                score_mms(psab, 1, pr2, start=False, stop=True)
                emit_reduces(pt, psab)
            h2 = pool_h.tile([128, 4 * CH], FP16, tag="h", name=f"h2_{t}")
            h2r = h2[:].rearrange("p (b c) -> p b c", b=4)
            nc.scalar.activation(
                h2r[0:64], cur1[0:64, None, :].to_broadcast([64, 4, CH]),
                relu, scale=1.0 / SC)
            nc.scalar.activation(
                h2r[64:128], cur1[0:64, None, :].to_broadcast([64, 4, CH]),
                relu, scale=1.0 / SC)
            nc.scalar.activation(d01[64:128, :], cur1[64:128, :], relu, scale=1.0 / SC)
            if t + 1 < NCHUNK:
                state = do_l0(t + 1, chunks[t + 1][1], chunks[t + 1][2])
            sab = ps_s.tile([33, 512], FP32, tag="sab", name=f"sab_{t}")
            score_mms(sab, 0, d01, start=True, stop=False)
            cur2 = layer_pass(t, 2, a_all, h2, fsb[1])
            r2 = pool_r2.tile([128, CH], FP16, tag="r2", name=f"r2_{t}")
            nc.scalar.activation(r2[:], cur2[:], relu, scale=1.0 / SC)
            del chunks[t]
            pending_reduce = (t, sab, r2)

        pt, psab, pr2 = pending_reduce
        score_mms(psab, 1, pr2, start=False, stop=True)
        emit_reduces(pt, psab)
        nc.vector.tensor_scalar_add(scores[:], scores[:], bias[0:1, 0:1])
        nc.sync.dma_start(out_d[:], scores[:])

    nc.compile()
    return nc


def _prep_inputs(nn_input, f0, f1, f2, w_nn, b_nn):
    """Host-side preprocessing into the kernel's layouts."""
    nn_input = np.asarray(nn_input, dtype=np.float32)
    f0 = np.asarray(f0, dtype=np.float32)
    f1 = np.asarray(f1, dtype=np.float32)
    f2 = np.asarray(f2, dtype=np.float32)
    w_nn = np.asarray(w_nn, dtype=np.float32).reshape(-1)
    b_nn = np.asarray(b_nn, dtype=np.float32).reshape(-1)

    # L1/L2 filters: [39*64, 128] i-major, x64, fp16 lhsT blocks
    def pack(f):
        out = np.zeros((NBLK * 128, 128), np.float32)
        out[: F0 * 64] = SC * f
        blocks = out.reshape(NBLK, 128, 128).transpose(1, 0, 2)
        return np.ascontiguousarray(blocks.reshape(128, NBLK * 128)).astype(
            np.float16
        )

    f1p, f2p = pack(f1), pack(f2)

    # L0: strictly-upper pairs packed; filter x2 (sym) x64, fp8 lhsT blocks
    iu, ju = np.triu_indices(F0, k=1)
    f0r = f0.reshape(F0, F0, 128)
    w0 = np.zeros((NP0 * 128, 128), np.float32)
    w0[: len(iu)] = 2.0 * SC * f0r[iu, ju]
    w0b = w0.reshape(NP0, 128, 128).transpose(1, 0, 2).reshape(128, NP0 * 128)
    f0p = np.ascontiguousarray(w0b).astype(ml_dtypes.float8_e4m3)

    wv = np.zeros((128, 2), np.float32)
    wv[0:64, 0] = 1.0 + w_nn[0:64]
    wv[64:128, 0] = 1.0 + w_nn[64:128]
    wv[:, 1] = 1.0 + w_nn[128:256]
    wv = wv.astype(np.float16)
    bias = b_nn.reshape(1, 1).astype(np.float32)

    x0 = nn_input.reshape(B, F0, D)
    in_maps = []
    for cidx in range(NCORES):
        xc = x0[cidx * BC : (cidx + 1) * BC]            # [BC, 39, 16]
        xt = xc.transpose(1, 0, 2).reshape(F0, N).astype(np.float16)
        x2h = np.zeros((40, N), np.float16)
        x2h[:F0] = xt
        xi = np.zeros((NP0 * 128, N), np.float16)
        xj = np.zeros((NP0 * 128, N), np.float16)
        xi[: len(iu)] = xt[iu]
        xj[: len(ju)] = xt[ju]
        in_maps.append(
            {"x2": x2h, "xi": xi, "xj": xj, "f0": f0p,
             "f1": f1p, "f2": f2p, "wv": wv, "bias": bias}
        )
    return in_maps


def _run(inputs, trace=False, trace_kwargs=None):
    global _BUILT
    if _BUILT is None:
        _BUILT = _build_program()
    nc = _BUILT
    in_maps = _prep_inputs(**inputs)
    res = run_bass_kernel_spmd(
        nc,
        in_maps,
        core_ids=list(range(NCORES)),
        trace=trace,
        **(trace_kwargs or {}),
    )
    out = np.concatenate(
        [res.results[c]["out"].reshape(BC) for c in range(NCORES)]
    )
    return out.reshape(B, 1).astype(np.float32), res


def kernel(**inputs):
    out, _ = _run(inputs)
    return out
